# revision 17
# baseline (speedup 1.0000x reference)
"""MoE (8 routed experts top-2 + shared expert) Trainium2 kernel.

Sharding (hardcoded, 8 cores): core c = (t, g) with t = c // 2 (token
quarter: 512 of 2048 tokens) and g = c % 2 (expert half: routed experts
4g..4g+3 plus columns [512g:512g+512] of the shared expert).  Each core
computes a partial output [D=1024, 512 tokens] (tokens on the matmul free
dim; x is fed pre-transposed), then the core pair (2t, 2t+1) ReduceScatters
along D (in two halves, the first overlapping compute) so each core ends
with two [256, 512] d-slices of the final output for its 512 tokens.  The
host reassembles/transposes the shards.

All matmuls run as float32r (full-rate fp32 on the PE, ~2.6e-4 rel err).
All DRAM inputs are host-prearranged so each DMA reads one contiguous
block per partition (128 descriptors/DMA instead of 1024+).
"""

import sys

sys.path.insert(0, "/opt/trn_rl_repo")

import numpy as np

import concourse.bass as bass
import concourse.tile as tile
import concourse.mybir as mybir
from concourse import bacc, masks
from concourse.bass_utils import run_bass_kernel_spmd

F32 = mybir.dt.float32
F32R = mybir.dt.float32r
ACT = mybir.ActivationFunctionType
ALU = mybir.AluOpType
AX = mybir.AxisListType

N_CORES = 8
D = 1024          # d_hidden
DE = 512          # d_expert (routed); also the shared-expert half width
E = 8             # routed experts
EL = 4            # routed experts per core
NE = EL + 1       # + shared-expert half
NT = 512          # tokens per core
DC = D // 128     # 8 contraction chunks of 128
HC = DE // 128    # 4 expert-width chunks of 128
NEG_BIG = -1.0e30


def build_program():
    nc = bacc.Bacc(num_devices=N_CORES)

    # ---- per-core DRAM I/O (all pre-permuted: partition dim first) ----
    xt_d = nc.dram_tensor("xt", [128, DC, NT], F32R, kind="ExternalInput")
    wgate_d = nc.dram_tensor("wgate", [128, DC, E], F32R, kind="ExternalInput")
    wg_d = nc.dram_tensor("wg", [NE, 128, DC, DE], F32R, kind="ExternalInput")
    wu_d = nc.dram_tensor("wu", [NE, 128, DC, DE], F32R, kind="ExternalInput")
    wd_d = nc.dram_tensor("wd", [DC, 128, NE * HC, 128], F32R, kind="ExternalInput")
    out_d = nc.dram_tensor("out", [2, 2, 128, NT], F32, kind="ExternalOutput")

    part_a = nc.dram_tensor("part_a", [4, 128, NT], F32)   # d[0:512]
    part_b = nc.dram_tensor("part_b", [4, 128, NT], F32)   # d[512:1024]
    rs_a = nc.dram_tensor("rs_a", [2, 128, NT], F32)
    rs_b = nc.dram_tensor("rs_b", [2, 128, NT], F32)

    with tile.TileContext(nc) as tc:
        with (
            tc.tile_pool(name="const", bufs=1) as constp,
            tc.tile_pool(name="xp", bufs=1) as xp,
            tc.tile_pool(name="gat", bufs=1) as gat,
            tc.tile_pool(name="wp", bufs=2) as wp,
            tc.tile_pool(name="hp", bufs=1) as hp,
            tc.tile_pool(name="sp", bufs=2) as sp,
            tc.tile_pool(name="wdp", bufs=3) as wdp,
            tc.tile_pool(name="ps", bufs=2, space="PSUM") as ps,
            tc.tile_pool(name="ps2", bufs=2, space="PSUM") as ps2,
        ):
            ident = constp.tile([128, 128], F32)
            masks.make_identity(nc, ident[:])

            # ---- input loads ----
            wgate_sb = xp.tile([128, DC, E], F32R)
            nc.sync.dma_start(wgate_sb[:], wgate_d[:])
            xt_sb = xp.tile([128, DC, NT], F32R)
            nc.sync.dma_start(xt_sb[:, 0:4, :], xt_d[:, 0:4, :])
            nc.sync.dma_start(xt_sb[:, 4:8, :], xt_d[:, 4:8, :])

            # ---- gating: logits in [e, n] layout ----
            ps_p = ps.tile([E, NT], F32, tag="ps_small")
            for c in range(DC):
                nc.tensor.matmul(
                    ps_p[:],
                    wgate_sb[:, c, :],
                    xt_sb[:, c, :],
                    start=(c == 0),
                    stop=(c == DC - 1),
                )
            logits_en = gat.tile([E, NT], F32)
            nc.vector.tensor_copy(logits_en[:], ps_p[:])

            # transpose to [n, e] (4 shots of [8, 128] -> [128, 8])
            p_ne = gat.tile([128, 4, E], F32)
            for q in range(4):
                tr_ps = ps.tile([128, E], F32, tag="ps_small")
                nc.tensor.transpose(
                    tr_ps[:], logits_en[:, q * 128 : (q + 1) * 128], ident[0:E, 0:E]
                )
                nc.vector.tensor_copy(p_ne[:, q, :], tr_ps[:])

            # softmax + top-2 mask (free-dim ops over e=8)
            m1 = gat.tile([128, 4], F32)
            nc.vector.tensor_reduce(m1[:], p_ne[:], axis=AX.X, op=ALU.max)
            m1b = m1[:].unsqueeze(2).broadcast_to((128, 4, E))
            eq1 = gat.tile([128, 4, E], F32)
            nc.vector.tensor_tensor(eq1[:], p_ne[:], m1b, op=ALU.is_equal)
            pm = gat.tile([128, 4, E], F32)
            nc.vector.scalar_tensor_tensor(
                pm[:], eq1[:], NEG_BIG, p_ne[:], op0=ALU.mult, op1=ALU.add
            )
            m2 = gat.tile([128, 4], F32)
            nc.vector.tensor_reduce(m2[:], pm[:], axis=AX.X, op=ALU.max)
            m2b = m2[:].unsqueeze(2).broadcast_to((128, 4, E))
            keep = gat.tile([128, 4, E], F32)
            nc.vector.tensor_tensor(keep[:], p_ne[:], m2b, op=ALU.is_ge)

            ex = gat.tile([128, 4, E], F32)
            nc.scalar.activation(ex[:], p_ne[:], ACT.Exp)
            ssum = gat.tile([128, 4], F32)
            nc.vector.tensor_reduce(ssum[:], ex[:], axis=AX.X, op=ALU.add)
            rec = gat.tile([128, 4], F32)
            nc.vector.reciprocal(rec[:], ssum[:])
            ek = gat.tile([128, 4, E], F32)
            nc.vector.tensor_tensor(ek[:], ex[:], keep[:], op=ALU.mult)
            recb = rec[:].unsqueeze(2).broadcast_to((128, 4, E))
            c_ne = gat.tile([128, 4, E], F32)
            nc.vector.tensor_tensor(c_ne[:], ek[:], recb, op=ALU.mult)

            # transpose back to [e, n] and broadcast my 4 experts' rows
            ps_ct = ps.tile([E, NT], F32, tag="ps_small")
            for q in range(4):
                nc.tensor.transpose(
                    ps_ct[:, q * 128 : (q + 1) * 128], c_ne[:, q, :], ident[:]
                )
            ct_sb = gat.tile([E, NT], F32)
            nc.vector.tensor_copy(ct_sb[:], ps_ct[:])
            cb = gat.tile([128, EL, NT], F32)
            for j in range(EL):
                crow = gat.tile([1, NT], F32, tag="crow")
                nc.gpsimd.dma_start(crow[0:1, :], ct_sb[j : j + 1, :])
                nc.gpsimd.partition_broadcast(cb[:, j, :], crow[0:1, :])

            # ---- up/gate + h for each expert (j=0..3 routed, j=4 shared) ----
            wd_tiles = {}

            def load_wd(dc):
                t = wdp.tile([128, NE * HC, 128], F32R, tag="wd")
                nc.sync.dma_start(t[:], wd_d[dc])
                wd_tiles[dc] = t

            for j in range(NE):
                wg_sb = wp.tile([128, DC, DE], F32R, tag="wg")
                nc.sync.dma_start(wg_sb[:, 0:4, :], wg_d[j, :, 0:4, :])
                wu_sb = wp.tile([128, DC, DE], F32R, tag="wu")
                nc.sync.dma_start(wu_sb[:, 0:4, :], wu_d[j, :, 0:4, :])
                nc.sync.dma_start(wg_sb[:, 4:8, :], wg_d[j, :, 4:8, :])
                nc.sync.dma_start(wu_sb[:, 4:8, :], wu_d[j, :, 4:8, :])

                h_sb = hp.tile([128, HC, NT], F32R, tag=f"h{j}")
                for hc in range(HC):
                    ps_g = ps2.tile([128, NT], F32, tag="ps_g")
                    ps_u = ps2.tile([128, NT], F32, tag="ps_u")
                    for c in range(DC):
                        nc.tensor.matmul(
                            ps_g[:],
                            wg_sb[:, c, hc * 128 : (hc + 1) * 128],
                            xt_sb[:, c, :],
                            start=(c == 0),
                            stop=(c == DC - 1),
                        )
                    for c in range(DC):
                        nc.tensor.matmul(
                            ps_u[:],
                            wu_sb[:, c, hc * 128 : (hc + 1) * 128],
                            xt_sb[:, c, :],
                            start=(c == 0),
                            stop=(c == DC - 1),
                        )
                    sil = sp.tile([128, NT], F32, tag="sil")
                    nc.scalar.activation(sil[:], ps_g[:], ACT.Silu)
                    if j < EL:
                        tt = sp.tile([128, NT], F32, tag="tt")
                        nc.vector.tensor_tensor(tt[:], sil[:], ps_u[:], op=ALU.mult)
                        nc.vector.tensor_tensor(
                            h_sb[:, hc, :], tt[:], cb[:, j, :], op=ALU.mult
                        )
                    else:
                        nc.vector.tensor_tensor(
                            h_sb[:, hc, :], sil[:], ps_u[:], op=ALU.mult
                        )
                if j == 3:
                    load_wd(0)
                if j == 4:
                    load_wd(1)
                if j == 0:
                    h0_sb = h_sb
                elif j == 1:
                    h1_sb = h_sb
                elif j == 2:
                    h2_sb = h_sb
                elif j == 3:
                    h3_sb = h_sb
                else:
                    h4_sb = h_sb
            h_all = [h0_sb, h1_sb, h2_sb, h3_sb, h4_sb]

            # ---- down projection: dc-outer, all experts accumulate in PSUM ----
            for dc in range(DC):
                if dc + 2 < DC:
                    load_wd(dc + 2)
                wd_sb = wd_tiles[dc]
                ps_o = ps2.tile([128, NT], F32, tag="ps_o")
                k = 0
                for j in range(NE):
                    for hc in range(HC):
                        nc.tensor.matmul(
                            ps_o[:],
                            wd_sb[:, j * HC + hc, :],
                            h_all[j][:, hc, :],
                            start=(k == 0),
                            stop=(k == NE * HC - 1),
                        )
                        k += 1
                ost = sp.tile([128, NT], F32, tag="ost")
                nc.vector.tensor_copy(ost[:], ps_o[:])
                if dc < 4:
                    nc.sync.dma_start(part_a[dc], ost[:])
                else:
                    nc.sync.dma_start(part_b[dc - 4], ost[:])
                if dc == 3:
                    nc.gpsimd.collective_compute(
                        "ReduceScatter",
                        ALU.add,
                        replica_groups=[[0, 1], [2, 3], [4, 5], [6, 7]],
                        ins=[part_a[:]],
                        outs=[rs_a[:]],
                    )
                    nc.sync.dma_start(out_d[0], rs_a[:])
            nc.gpsimd.collective_compute(
                "ReduceScatter",
                ALU.add,
                replica_groups=[[0, 1], [2, 3], [4, 5], [6, 7]],
                ins=[part_b[:]],
                outs=[rs_b[:]],
            )
            nc.sync.dma_start(out_d[1], rs_b[:])

    nc.compile()
    return nc


_NC_CACHE = None


def _get_program():
    global _NC_CACHE
    if _NC_CACHE is None:
        _NC_CACHE = build_program()
    return _NC_CACHE


def _perm_rows(m):
    """[1024, X] -> [128, 8, X] with row (c*128+p) at [p, c]."""
    return np.ascontiguousarray(
        m.reshape(DC, 128, -1).transpose(1, 0, 2)
    )


def _make_in_maps(x, W_g, Wg_e, Wu_e, Wd_e, Wg_s, Wu_s, Wd_s):
    xf = np.asarray(x, dtype=np.float32).reshape(2048, D)
    xT = np.ascontiguousarray(xf.T)  # [D, 2048]
    W_g = np.asarray(W_g, dtype=np.float32)
    Wg_e = np.asarray(Wg_e, dtype=np.float32)
    Wu_e = np.asarray(Wu_e, dtype=np.float32)
    Wd_e = np.asarray(Wd_e, dtype=np.float32)
    Wg_s = np.asarray(Wg_s, dtype=np.float32)
    Wu_s = np.asarray(Wu_s, dtype=np.float32)
    Wd_s = np.asarray(Wd_s, dtype=np.float32)

    in_maps = []
    for g in range(2):
        order = list(range(4 * g, 4 * g + 4)) + list(range(4 * (1 - g), 4 * (1 - g) + 4))
        wgate = _perm_rows(np.ascontiguousarray(W_g[:, order]))
        wg = np.stack(
            [_perm_rows(Wg_e[e]) for e in range(4 * g, 4 * g + 4)]
            + [_perm_rows(Wg_s[:, 512 * g : 512 * g + 512])]
        )
        wu = np.stack(
            [_perm_rows(Wu_e[e]) for e in range(4 * g, 4 * g + 4)]
            + [_perm_rows(Wu_s[:, 512 * g : 512 * g + 512])]
        )
        # wd: [DC, 128, NE*HC, 128]; [dc, p, j*HC+hc, dcol] = Wd_j[hc*128+p, dc*128+dcol]
        wd_stack = np.stack(
            [Wd_e[e] for e in range(4 * g, 4 * g + 4)]
            + [Wd_s[512 * g : 512 * g + 512, :]]
        )  # [NE, DE, D]
        wd = np.ascontiguousarray(
            wd_stack.reshape(NE, HC, 128, DC, 128).transpose(3, 2, 0, 1, 4)
        )  # [DC, 128, NE, HC, 128]
        wd = wd.reshape(DC, 128, NE * HC, 128)
        for_g = (wgate, wg, wu, wd)
        if g == 0:
            g0 = for_g
        else:
            g1 = for_g

    for c in range(N_CORES):
        t, g = c // 2, c % 2
        wgate, wg, wu, wd = g0 if g == 0 else g1
        in_maps.append(
            {
                "xt": _perm_rows(np.ascontiguousarray(xT[:, t * NT : (t + 1) * NT])),
                "wgate": wgate,
                "wg": wg,
                "wu": wu,
                "wd": wd,
            }
        )
    return in_maps


def kernel(x, W_g, Wg_e, Wu_e, Wd_e, Wg_s, Wu_s, Wd_s, _trace=False, _trace_kwargs=None):
    nc = _get_program()
    in_maps = _make_in_maps(x, W_g, Wg_e, Wu_e, Wd_e, Wg_s, Wu_s, Wd_s)
    res = run_bass_kernel_spmd(
        nc, in_maps, list(range(N_CORES)), trace=_trace, **(_trace_kwargs or {})
    )

    # out_d [2(half), 2(rank-slice), 128, NT]; core c=(t, r):
    #   d = half*512 + r*256 + q*128 + p  for piece [half, q, p, :]
    out = np.empty((2048, D), dtype=np.float32)
    for t in range(4):
        for r in range(2):
            o = res.results[2 * t + r]["out"]  # [2, 2, 128, NT]
            for half in range(2):
                d0 = half * 512 + r * 256
                blk = o[half].reshape(256, NT)  # [d0:d0+256, tokens]
                out[t * NT : (t + 1) * NT, d0 : d0 + 256] = blk.T
    result = out.reshape(2, 1024, D)
    if _trace:
        return result, res
    return result
